# revision 66
# baseline (speedup 1.0000x reference)
"""Trainium2 Bass kernel for MemexQA-FVTA (dense transformer block), v2.

Data-parallel over batch across 8 NeuronCores (8 elems/core, no collectives).

Key differences vs v1 baseline:
- Length-aware tiling: batch elems are sorted by valid token-tile count
  (ceil(text_len/128) + 1 img tile), dealt into 8 per-core slots; each
  slot's tile count is baked into the compiled program (keyed compile
  cache), skipping ~40% of the MACs on average inputs.
- Transposed self-attention softmax: scores computed key-major so the pad
  mask becomes a free per-partition ACT bias and the softmax transposes
  disappear; normalization is deferred (unnormalized album scaled by a
  broadcast 1/rowsum).
- All weights resident in SBUF (no per-elem weight re-DMA); album/keys/
  vals path in bf16 (same PE rate, half the SBUF/DVE traffic).
- LayerNorm rstd via DVE Newton-rsqrt (bitcast seed) - no activation
  table thrash (v1 lost 82us to Ln/Exp table swaps).
- Biases folded: q/k as ACT bias, v as rank-1 PSUM matmuls, bkp as ACT
  bias, bvp applied host-side (exact: sum of FVTA mean weights = ql/LQ).
"""

import sys
import numpy as np

H, B, LT, LI, LQ = 2, 64, 384, 128, 24
D, KD, VD = 768, 384, 384
DV = H * VD          # 768
NCORES = 8
BL = B // NCORES     # 8 batch elements per core
NKC = D // 128       # 6 contraction chunks
MASK = -30000.0
EPS = 1e-5
MAGIC = 0x5F3759DF


def _ensure_path():
    try:
        import concourse  # noqa: F401
    except ImportError:
        sys.path.insert(0, "/opt/trn_rl_repo")


_COMPILED = {}


def build_nc(chunks):
    """Build + compile the per-core Bass program for the given per-slot
    token-tile counts (tuple of 8 ints, each in 2..4). Cached."""
    global _COMPILED
    chunks = tuple(int(c) for c in chunks)
    if chunks in _COMPILED:
        return _COMPILED[chunks]
    _ensure_path()
    from contextlib import ExitStack

    import concourse.bacc as bacc
    import concourse.tile as tile
    import concourse.mybir as mybir

    f32 = mybir.dt.float32
    f32r = mybir.dt.float32r
    bf16 = mybir.dt.bfloat16
    i32 = mybir.dt.int32
    AF = mybir.ActivationFunctionType
    ALU = mybir.AluOpType
    AX = mybir.AxisListType

    TOTC = sum(chunks)
    OFF = [0]
    for c in chunks:
        OFF.append(OFF[-1] + c)

    nc = bacc.Bacc("TRN2", target_bir_lowering=False, debug=False,
                   num_devices=NCORES)

    def din(name, shape, dt=None):
        return nc.declare_dram_parameter(name, list(shape), dt or f32r, False).ap()

    text_d = din("text", [BL, LT, D])
    images_d = din("images", [BL, LI, D])
    query_d = din("query", [BL, LQ, D])
    # packed proj weights: [H, 128, NKC*384]; [h, p, kc*384+m] = W[h, kc*128+p, m]
    w_d = {n: din(n, [H, 128, NKC * 384])
           for n in ["wsq", "wiq", "wsk", "wik", "wsv", "wiv"]}
    wkp_d = din("wkp", [128, NKC * DV], bf16)   # [p, dkc*768+vd] = Wkp[vd,dkc*128+p]
    wvp_d = din("wvp", [128, 12 * 384], bf16)   # [p,(vdc*2+hf)*384+m]
    bqk_d = din("bqk", [128, 24], f32)          # col=(qk*2+h)*3+mf (+12 img)
    bkp_d = din("bkp_t", [128, 6], bf16)
    bv_si_d = din("bv_si", [1, 4 * VD])         # seg = si*2 + h
    kmT_d = din("kmT", [128, TOTC], f32)        # per-partition key mask cols
    qvalid_d = din("qvalid", [1, BL * LQ], f32)  # [j*LQ+q]: valid/LQ, slot j
    ident_d = din("ident_r", [128, 128])
    ones_d = din("ones_r", [1, 128])
    out_d = nc.declare_dram_parameter("out", [BL, DV], f32, True).ap()

    with tile.TileContext(nc) as tc, ExitStack() as ctx:
        def pool(**kw):
            return ctx.enter_context(tc.tile_pool(**kw))

        cpool = pool(name="const", bufs=1)
        wpool = pool(name="wres", bufs=1)
        lnp = pool(name="ln", bufs=2)
        scrq = pool(name="scrq", bufs=1)
        scr = pool(name="scr", bufs=1)
        stat = pool(name="stat", bufs=3)
        xtp = pool(name="xt", bufs=1)
        qkp = pool(name="qk", bufs=1)
        vp = pool(name="v", bufs=1)
        etp = pool(name="et", bufs=1)
        rbp = pool(name="rb", bufs=2)
        sinvp = pool(name="sinv", bufs=1)
        albp = pool(name="alb", bufs=1)
        qkpp = pool(name="qkp24", bufs=1)
        qftp = pool(name="qft", bufs=2)
        smallp = pool(name="sml", bufs=1)
        pmm = pool(name="pmm", bufs=4, space="PSUM")
        ptr = pool(name="ptr", bufs=2, space="PSUM")
        psml = pool(name="psml", bufs=2, space="PSUM")

        def r(ap):
            return ap.bitcast(f32r)

        def p(ap):
            return ap.bitcast(f32)

        # ---- constants ----
        ident = cpool.tile([128, 128], f32r, tag="ident")
        nc.sync.dma_start(ident[:], ident_d[:])
        ones1 = cpool.tile([1, 128], f32r, tag="ones1")
        nc.sync.dma_start(ones1[:], ones_d[:])
        negI = cpool.tile([128, 128], f32, tag="negI")
        nc.gpsimd.memset(negI[:], 0.0)
        nc.gpsimd.affine_select(
            out=negI[:], in_=negI[:], compare_op=ALU.not_equal, fill=MASK,
            base=0, pattern=[[-1, 128]], channel_multiplier=1)
        ones_bf = cpool.tile([128, 1], bf16, tag="onesbf")
        nc.gpsimd.memset(ones_bf[:], 1.0)
        bqk = cpool.tile([128, 24], f32, tag="bqk")
        nc.sync.dma_start(bqk[:], bqk_d[:])
        bkp = cpool.tile([128, 6], bf16, tag="bkp")
        nc.sync.dma_start(bkp[:], bkp_d[:])
        bv_si = cpool.tile([1, 4 * VD], f32r, tag="bvsi")
        nc.sync.dma_start(bv_si[:], bv_si_d[:])
        kmT = cpool.tile([128, TOTC], f32, tag="kmT")
        nc.sync.dma_start(kmT[:], kmT_d[:])
        qvalid = cpool.tile([1, BL * LQ], f32, tag="qvalid")
        nc.sync.dma_start(qvalid[:], qvalid_d[:])

        wt = {}

        # ================= per-elem pipeline =================
        def preamble(j):
            """DMA + LN + transposes for slot j. Returns (qfT, xT)."""
            C = chunks[j]
            N = C * 128
            qtile = scrq.tile([LQ, D], f32r, tag="q")
            nc.sync.dma_start(qtile[:], query_d[j])
            qfT = qftp.tile([128, NKC * LQ], bf16, tag="qfT")
            for kc in range(NKC):
                pt = ptr.tile([128, 128], f32r, tag="tr")
                nc.tensor.transpose(r(pt[:, 0:LQ]),
                                    r(qtile[:, kc * 128:(kc + 1) * 128]),
                                    r(ident[0:LQ, 0:LQ]))
                nc.vector.tensor_copy(qfT[:, kc * LQ:(kc + 1) * LQ],
                                      p(pt[:, 0:LQ]))
            # qk_proj = qf @ Wkp.T [24, 768] and qbias = qf @ bkp [1, 24]
            # (fvta inputs that only need qfT - computed early, off the
            # fvta critical path)
            qkp24 = qkpp.tile([LQ, DV], f32r, tag="qkp24")
            ps_h0 = pmm.tile([128, 512], f32, tag="mm")
            ps_h1 = pmm.tile([128, 512], f32, tag="mm")
            pss = [ps_h0, ps_h1]
            for dkc in range(NKC):
                for hf in range(2):
                    nc.tensor.matmul(
                        pss[hf][0:LQ, 0:384],
                        qfT[:, dkc * LQ:(dkc + 1) * LQ],
                        wkp_s[:, dkc * DV + hf * 384: dkc * DV + hf * 384 + 384],
                        start=(dkc == 0), stop=(dkc == NKC - 1))
            for hf in range(2):
                nc.vector.tensor_copy(qkp24[:, hf * 384:(hf + 1) * 384],
                                      pss[hf][0:LQ, 0:384])
            qkpT = qftp.tile([128, NKC * LQ], bf16, tag="qkpT")
            for vdc in range(NKC):
                pt = ptr.tile([128, 128], f32r, tag="tr")
                nc.tensor.transpose(r(pt[:, 0:LQ]),
                                    r(qkp24[:, vdc * 128:(vdc + 1) * 128]),
                                    r(ident[0:LQ, 0:LQ]))
                nc.vector.tensor_copy(qkpT[:, vdc * LQ:(vdc + 1) * LQ],
                                      p(pt[:, 0:LQ]))
            psQ = psml.tile([128, 512], f32, tag="sml")
            for dkc in range(NKC):
                nc.tensor.matmul(psQ[0:1, 0:LQ], bkp[:, dkc:dkc + 1],
                                 qfT[:, dkc * LQ:(dkc + 1) * LQ],
                                 start=(dkc == 0), stop=(dkc == NKC - 1))
            qbrow = qftp.tile([1, LQ], f32r, tag="qbrow")
            nc.vector.tensor_copy(qbrow[0:1, :], psQ[0:1, 0:LQ])
            xT = xtp.tile([128, NKC * 512], f32r, tag="xT")
            for tt in range(C):
                if tt < C - 1:
                    src = text_d[j, tt * 128:(tt + 1) * 128, :]
                else:
                    src = images_d[j]
                x = lnp.tile([128, D], f32r, tag="x")
                nc.sync.dma_start(x[:], src)
                bnst = stat.tile([128, 12], f32, tag="bnst")
                nc.vector.bn_stats(bnst[:, 0:6], p(x[:, 0:384]))
                nc.vector.bn_stats(bnst[:, 6:12], p(x[:, 384:768]))
                mv = stat.tile([128, 2], f32, tag="mv")
                nc.vector.bn_aggr(mv[:], bnst[:])
                mu = mv[:, 0:1]
                ve = stat.tile([128, 1], f32, tag="ve")
                nc.gpsimd.tensor_scalar_add(ve[:], mv[:, 1:2], EPS)
                # Newton rsqrt: seed from bitcast magic, 2 iterations
                t1 = stat.tile([128, 1], i32, tag="t1")
                nc.vector.tensor_scalar(t1[:], ve[:].bitcast(i32), 1, None,
                                        ALU.logical_shift_right)
                t2 = stat.tile([128, 1], i32, tag="t2")
                nc.vector.tensor_scalar_mul(t2[:], t1[:], -1)
                nc.vector.tensor_scalar_add(t2[:], t2[:], MAGIC)
                y0 = t2[:].bitcast(f32)
                hh = stat.tile([128, 1], f32, tag="hh")
                nc.vector.tensor_scalar_mul(hh[:], ve[:], -0.5)
                z = stat.tile([128, 1], f32, tag="z")
                y1 = stat.tile([128, 1], f32, tag="y1")
                nc.vector.tensor_mul(z[:], y0, y0)
                nc.vector.tensor_mul(z[:], z[:], hh[:])
                nc.vector.tensor_scalar_add(z[:], z[:], 1.5)
                nc.vector.tensor_mul(y1[:], y0, z[:])
                rstd = stat.tile([128, 1], f32, tag="rstd")
                nc.vector.tensor_mul(z[:], y1[:], y1[:])
                nc.vector.tensor_mul(z[:], z[:], hh[:])
                nc.vector.tensor_scalar_add(z[:], z[:], 1.5)
                nc.vector.tensor_mul(rstd[:], y1[:], z[:])
                nma = stat.tile([128, 1], f32, tag="nma")
                nc.gpsimd.tensor_mul(nma[:], mu, rstd[:])
                nmr = stat.tile([128, 1], f32, tag="nmr")
                nc.gpsimd.tensor_scalar_mul(nmr[:], nma[:], -1.0)
                nc.scalar.activation(x[:], p(x[:]), AF.Identity,
                                     bias=nmr[:, 0:1], scale=rstd[:, 0:1])
                for kc in range(NKC):
                    pt = ptr.tile([128, 128], f32r, tag="tr")
                    nc.tensor.transpose(r(pt[:]),
                                        r(x[:, kc * 128:(kc + 1) * 128]),
                                        r(ident[:]))
                    nc.vector.tensor_copy(
                        xT[:, kc * N + tt * 128: kc * N + tt * 128 + 128],
                        p(pt[:]))
            return qkpT, qbrow, xT

        def compute_heads(j, state):
            """q/k/v projections + transposed-softmax attention for slot j.
            Returns bf16 albumT [128, 6*N] (head-concat, 1/s-scaled)."""
            qkpT, qbrow, xT = state
            C = chunks[j]
            N = C * 128
            NTX = (C - 1) * 128
            albumT = albp.tile([128, NKC * 512], bf16, tag="albumT")
            for h in range(H):
                qT = qkp.tile([128, 3 * 512], f32r, tag="qT")
                kT = qkp.tile([128, 3 * 512], f32r, tag="kT")
                for qk, dstT in ((0, qT), (1, kT)):
                    wS = wt[("wsq" if qk == 0 else "wsk", h)]
                    wI = wt[("wiq" if qk == 0 else "wik", h)]
                    if NTX >= 256:
                        # text tokens: feature-major direct (full-rate N)
                        for mf in range(3):
                            ps = pmm.tile([128, 512], f32, tag="mm")
                            for kc in range(NKC):
                                nc.tensor.matmul(
                                    ps[:, 0:NTX],
                                    r(wS[:, kc * 384 + mf * 128:
                                         kc * 384 + mf * 128 + 128]),
                                    r(xT[:, kc * N: kc * N + NTX]),
                                    start=(kc == 0), stop=(kc == NKC - 1))
                            bcol = (qk * 2 + h) * 3 + mf
                            nc.scalar.activation(
                                dstT[:, mf * N: mf * N + NTX],
                                ps[:, 0:NTX], AF.Identity,
                                bias=bqk[:, bcol:bcol + 1])
                        text_tcs = []
                    else:
                        # single text tile: token-major (N=384) + transpose
                        text_tcs = list(range(C - 1))
                    for tc in text_tcs + [C - 1]:
                        istext = tc < C - 1
                        wX = wS if istext else wI
                        ps2 = pmm.tile([128, 512], f32, tag="mm")
                        for kc in range(NKC):
                            nc.tensor.matmul(
                                ps2[:, 0:KD],
                                r(xT[:, kc * N + tc * 128:
                                     kc * N + tc * 128 + 128]),
                                r(wX[:, kc * 384:(kc + 1) * 384]),
                                start=(kc == 0), stop=(kc == NKC - 1))
                        tm = scr.tile([128, KD], f32r, tag="imgtm")
                        nc.vector.tensor_copy(tm[:], ps2[:, 0:KD])
                        for mf in range(3):
                            pt = ptr.tile([128, 128], f32r, tag="tr")
                            nc.tensor.transpose(
                                r(pt[:]), r(tm[:, mf * 128:(mf + 1) * 128]),
                                r(ident[:]))
                            bcol = (0 if istext else 12) + (qk * 2 + h) * 3 + mf
                            nc.scalar.activation(
                                dstT[:, mf * N + tc * 128:
                                     mf * N + tc * 128 + 128], p(pt[:]),
                                AF.Identity, bias=bqk[:, bcol:bcol + 1])
                # v: token-major
                v = vp.tile([128, 4 * VD], bf16, tag="v")
                for tc in range(C):
                    istext = tc < C - 1
                    wV = wt[("wsv" if istext else "wiv", h)]
                    ps = pmm.tile([128, 512], f32, tag="mm")
                    for kc in range(NKC):
                        nc.tensor.matmul(
                            ps[:, 0:VD],
                            r(xT[:, kc * N + tc * 128: kc * N + tc * 128 + 128]),
                            r(wV[:, kc * 384:(kc + 1) * 384]),
                            start=(kc == 0), stop=False)
                    seg = (0 if istext else 2) + h
                    nc.tensor.matmul(ps[:, 0:VD], r(ones1[0:1, 0:128]),
                                     r(bv_si[0:1, seg * VD:(seg + 1) * VD]),
                                     start=False, stop=True)
                    nc.vector.tensor_copy(v[:, tc * VD:(tc + 1) * VD],
                                          ps[:, 0:VD])
                # transposed scores + masked softmax (unnormalized)
                eT = etp.tile([128, 4 * 512], bf16, tag="eT")
                psS = psml.tile([128, 512], f32, tag="sml")
                for kc in range(C):
                    ps = pmm.tile([128, 512], f32, tag="mm")
                    for mf in range(3):
                        nc.tensor.matmul(
                            ps[:, 0:N],
                            r(kT[:, mf * N + kc * 128: mf * N + kc * 128 + 128]),
                            r(qT[:, mf * N:(mf + 1) * N]),
                            start=(mf == 0), stop=(mf == 2))
                    nc.vector.tensor_add(ps[:, kc * 128:(kc + 1) * 128],
                                         ps[:, kc * 128:(kc + 1) * 128],
                                         negI[:])
                    nc.scalar.activation(eT[:, kc * N:(kc + 1) * N],
                                         ps[:, 0:N], AF.Exp,
                                         bias=kmT[:, OFF[j] + kc: OFF[j] + kc + 1])
                    nc.tensor.matmul(psS[0:1, 0:N], ones_bf[:, 0:1],
                                     eT[:, kc * N:(kc + 1) * N],
                                     start=(kc == 0), stop=(kc == C - 1))
                srow = sinvp.tile([1, 512], f32r, tag="srow")
                nc.vector.tensor_copy(srow[0:1, 0:N], psS[0:1, 0:N])
                psR = psml.tile([128, 512], f32, tag="sml")
                nc.tensor.matmul(psR[:, 0:N], r(ones1[0:1, 0:128]),
                                 srow[0:1, 0:N], start=True, stop=True)
                rb = rbp.tile([128, 512], f32, tag="rb")
                nc.vector.reciprocal(rb[:, 0:N], psR[:, 0:N])
                for vdc in range(3):
                    psA = pmm.tile([128, 512], f32, tag="mm")
                    for kc in range(C):
                        nc.tensor.matmul(
                            psA[:, 0:N],
                            v[:, kc * VD + vdc * 128: kc * VD + vdc * 128 + 128],
                            eT[:, kc * N:(kc + 1) * N],
                            start=(kc == 0), stop=(kc == C - 1))
                    nc.vector.tensor_mul(
                        albumT[:, (h * 3 + vdc) * N:(h * 3 + vdc + 1) * N],
                        psA[:, 0:N], rb[:, 0:N])
            return albumT

        def fvta(j, albumT, qkpT, qbrow):
            C = chunks[j]
            N = C * 128
            # w2 key-major: [t, q]; pad mask is a per-partition exp bias
            e2T = smallp.tile([128, 4 * LQ], bf16, tag="e2T")
            psS2 = psml.tile([128, 512], f32, tag="sml")
            for tc in range(C):
                psw = ptr.tile([128, 128], f32r, tag="tr")
                for vdc in range(NKC):
                    nc.tensor.matmul(
                        p(psw[:, 0:LQ]),
                        albumT[:, vdc * N + tc * 128: vdc * N + tc * 128 + 128],
                        qkpT[:, vdc * LQ:(vdc + 1) * LQ],
                        start=(vdc == 0), stop=False)
                nc.tensor.matmul(p(psw[:, 0:LQ]), r(ones1[0:1, 0:128]),
                                 qbrow[0:1, :], start=False, stop=True)
                nc.scalar.activation(e2T[:, tc * LQ:(tc + 1) * LQ],
                                     p(psw[:, 0:LQ]), AF.Exp,
                                     bias=kmT[:, OFF[j] + tc: OFF[j] + tc + 1])
                nc.tensor.matmul(psS2[0:1, 0:LQ], ones_bf[:, 0:1],
                                 e2T[:, tc * LQ:(tc + 1) * LQ],
                                 start=(tc == 0), stop=(tc == C - 1))
            r2 = smallp.tile([1, LQ], f32, tag="r2")
            nc.vector.reciprocal(r2[0:1, :], psS2[0:1, 0:LQ])
            wp_row = smallp.tile([1, LQ], f32r, tag="wprow")
            nc.vector.tensor_mul(wp_row[0:1, :],
                                 qvalid[0:1, j * LQ:(j + 1) * LQ],
                                 r2[0:1, :])
            pswb = psml.tile([128, 512], f32, tag="sml")
            nc.tensor.matmul(pswb[:, 0:LQ], r(ones1[0:1, 0:128]),
                             wp_row[0:1, :], start=True, stop=True)
            wpB = smallp.tile([128, LQ], f32, tag="wpB")
            nc.vector.tensor_copy(wpB[:], pswb[:, 0:LQ])
            macc = smallp.tile([128, 4], f32, tag="macc")
            mtmp = smallp.tile([128, LQ], f32, tag="mtmp")
            for tc in range(C):
                nc.vector.tensor_mul(mtmp[:], e2T[:, tc * LQ:(tc + 1) * LQ],
                                     wpB[:])
                nc.vector.reduce_sum(macc[:, tc:tc + 1], mtmp[:], axis=AX.X)
            maccr = smallp.tile([128, 4], f32r, tag="maccr")
            nc.vector.tensor_copy(maccr[:, 0:C], macc[:, 0:C])
            # mbar row -> broadcast; zvec = album_sc @ mbar (DVE mul+reduce)
            mrow = smallp.tile([1, 512], f32r, tag="mrow")
            for tc in range(C):
                pt = ptr.tile([128, 128], f32r, tag="tr")
                nc.tensor.transpose(r(pt[0:1, :]), r(maccr[:, tc:tc + 1]),
                                    r(ident[:]))
                nc.vector.tensor_copy(mrow[0:1, tc * 128:(tc + 1) * 128],
                                      p(pt[0:1, :]))
            psB = psml.tile([128, 512], f32, tag="sml")
            nc.tensor.matmul(psB[:, 0:N], r(ones1[0:1, 0:128]),
                             mrow[0:1, 0:N], start=True, stop=True)
            zv = smallp.tile([128, NKC], f32, tag="zv")
            for vdc in range(NKC):
                zscr = rbp.tile([128, 512], f32, tag="zscr")
                nc.vector.tensor_mul(zscr[:, 0:N], albumT[:, vdc * N:(vdc + 1) * N],
                                     psB[:, 0:N])
                nc.vector.reduce_sum(zv[:, vdc:vdc + 1], zscr[:, 0:N], axis=AX.X)
            zbf = smallp.tile([128, NKC], bf16, tag="zbf")
            nc.vector.tensor_copy(zbf[:], zv[:])
            for hf in range(2):
                pso = psml.tile([128, 512], f32, tag="sml")
                for vdc in range(NKC):
                    nc.tensor.matmul(
                        pso[0:1, 0:384], zbf[:, vdc:vdc + 1],
                        wvp_s[:, (vdc * 2 + hf) * 384:
                              (vdc * 2 + hf + 1) * 384],
                        start=(vdc == 0), stop=(vdc == NKC - 1))
                outrow = smallp.tile([1, 384], f32, tag="outrow")
                nc.vector.tensor_copy(outrow[0:1, :], pso[0:1, 0:384])
                nc.sync.dma_start(out_d[j:j + 1, hf * 384:(hf + 1) * 384],
                                  outrow[0:1, :])

        # PE warmup: keep the HAM activity window busy while input/weight
        # DMAs land, so the first real matmuls run at 2.4 GHz
        for _ in range(24):
            ptw = ptr.tile([128, 128], f32r, tag="tr")
            nc.tensor.transpose(r(ptw[:]), r(ident[:]), r(ident[:]))

        wkp_s = wpool.tile([128, NKC * DV], bf16, tag="wkp")
        nc.sync.dma_start(wkp_s[:], wkp_d[:])

        state = preamble(0)

        # resident weights emitted after slot-0 preamble so its input DMAs
        # aren't queued behind ~17MB of weights; h-major, q/k first
        for h in range(H):
            for name in ["wsq", "wsk", "wiq", "wik", "wsv", "wiv"]:
                t = wpool.tile([128, NKC * 384], f32r, tag=f"{name}{h}")
                nc.sync.dma_start(t[:], w_d[name][h])
                wt[(name, h)] = t
        wvp_s = wpool.tile([128, 12 * 384], bf16, tag="wvp")
        nc.sync.dma_start(wvp_s[:], wvp_d[:])

        for j in range(BL):
            qkpT, qbrow, xT = state
            albumT = compute_heads(j, state)
            if j + 1 < BL:
                state = preamble(j + 1)
            fvta(j, albumT, qkpT, qbrow)

    nc.compile()
    _COMPILED[chunks] = nc
    return nc


def plan_slots(text_lengths):
    """Sort elems by token-tile count desc, deal into 8 slots x 8 cores.
    Returns (order[64], chunks[8]): core c slot j processes elem
    order[8*j + c]; chunks[j] = baked tile count for slot j."""
    tl = np.asarray(text_lengths)
    ntt = np.clip(np.ceil(tl / 128).astype(np.int64), 1, 3)
    order = np.argsort(-ntt, kind="stable")
    chunks = [int(ntt[order[8 * jj]]) + 1 for jj in range(BL)]
    return order, chunks


def make_in_maps(text, images, query, ln_gamma, ln_beta,
                 Wsq, bsq, Wiq, biq, Wsk, bsk, Wik, bik, Wsv, bsv, Wiv, biv,
                 Wkp, bkp, Wvp, bvp,
                 text_lengths, image_lengths, query_lengths):
    """Host-side preprocessing + slot-sorted batch sharding."""
    _ensure_path()
    import ml_dtypes
    f = np.float32
    g = np.asarray(ln_gamma, f)
    beta = np.asarray(ln_beta, f)

    order, chunks = plan_slots(text_lengths)
    TOTC = sum(chunks)
    OFF = [0]
    for c in chunks:
        OFF.append(OFF[-1] + c)

    def fold_w(W):
        return np.asarray(W, f) * g[None, :, None]

    def pack_w(W):
        M = W.shape[2]
        return np.ascontiguousarray(
            W.reshape(H, NKC, 128, M).transpose(0, 2, 1, 3).reshape(H, 128, NKC * M))

    def beta_bias(W, bias):
        Wf = fold_w(W)
        return (np.einsum("d,hdm->hm", beta, Wf) + np.asarray(bias, f)).astype(f)

    ws = {}
    for name, W in [("wsq", Wsq), ("wiq", Wiq), ("wsk", Wsk), ("wik", Wik),
                    ("wsv", Wsv), ("wiv", Wiv)]:
        ws[name] = pack_w(fold_w(W))
    bq_s = beta_bias(Wsq, bsq)
    bk_s = beta_bias(Wsk, bsk)
    bv_s = beta_bias(Wsv, bsv)
    bq_i = beta_bias(Wiq, biq)
    bk_i = beta_bias(Wik, bik)
    bv_i = beta_bias(Wiv, biv)

    # bqk [128, 24]: col=(qk*2+h)*3+mf text, +12 img
    bqk = np.zeros((128, 24), f)
    for qk, (bt, bi) in enumerate([(bq_s, bq_i), (bk_s, bk_i)]):
        for h in range(H):
            for mf in range(3):
                col = (qk * 2 + h) * 3 + mf
                bqk[:, col] = bt[h, mf * 128:(mf + 1) * 128]
                bqk[:, 12 + col] = bi[h, mf * 128:(mf + 1) * 128]
    bv_si = np.concatenate([bv_s[0], bv_s[1], bv_i[0], bv_i[1]]).astype(f)
    bv_si = bv_si.reshape(1, 4 * VD)

    Wkp_ = np.asarray(Wkp, f)
    # [p, dkc*768 + vd] = Wkp[vd, dkc*128+p]
    wkp_p = np.ascontiguousarray(
        Wkp_.reshape(DV, NKC, 128).transpose(2, 1, 0).reshape(128, NKC * DV))
    Wvp_ = np.asarray(Wvp, f)
    wvp_p = np.ascontiguousarray(
        Wvp_.reshape(NKC, 128, 2, 384).transpose(1, 0, 2, 3).reshape(128, 12 * 384))
    bkp_t = np.ascontiguousarray(np.asarray(bkp, f).reshape(6, 128).T)

    tl = np.asarray(text_lengths)
    il = np.asarray(image_lengths)
    ql = np.asarray(query_lengths)

    def rnd(a):
        a = np.ascontiguousarray(np.asarray(a, f))
        return (a.view(np.uint32) & np.uint32(0xFFFFF000)).view(np.float32)

    ident_r = rnd(np.eye(128, dtype=f))
    ones_r = rnd(np.ones((1, 128), f))
    text = rnd(np.asarray(text, f))
    images = rnd(np.asarray(images, f))
    query = rnd(np.asarray(query, f))
    for n in list(ws):
        ws[n] = rnd(ws[n])
    bv_si = rnd(bv_si)
    wkp_b = wkp_p.astype(ml_dtypes.bfloat16)
    wvp_b = wvp_p.astype(ml_dtypes.bfloat16)
    bkp_b = bkp_t.astype(ml_dtypes.bfloat16)

    in_maps = []
    for c in range(NCORES):
        el = [int(order[8 * jj + c]) for jj in range(BL)]
        kmT = np.zeros((128, TOTC), f)
        qvalid_r = np.zeros((1, BL * LQ), f)
        for jj in range(BL):
            e = el[jj]
            C = chunks[jj]
            km = np.zeros(C * 128, f)
            ntx = (C - 1) * 128
            km[:ntx][np.arange(ntx) >= tl[e]] = MASK
            km[ntx:][np.arange(128) >= il[e]] = MASK
            for cc in range(C):
                kmT[:, OFF[jj] + cc] = km[cc * 128:(cc + 1) * 128]
            qvalid_r[0, jj * LQ:(jj + 1) * LQ] = \
                (np.arange(LQ) < ql[e]).astype(f) / LQ
        in_maps.append({
            "text": np.ascontiguousarray(text[el]),
            "images": np.ascontiguousarray(images[el]),
            "query": np.ascontiguousarray(query[el]),
            **{n: ws[n] for n in ws},
            "wkp": wkp_b, "wvp": wvp_b,
            "bqk": bqk, "bkp_t": bkp_b, "bv_si": bv_si,
            "kmT": kmT, "qvalid": qvalid_r,
            "ident_r": ident_r, "ones_r": ones_r,
        })
    return in_maps, order, chunks


def run(in_maps, chunks, trace=False, tmpdir=None):
    _ensure_path()
    from concourse import bass_utils
    nc = build_nc(chunks)
    kw = {}
    if trace:
        kw = dict(trace=True, tmpdir=tmpdir)
    res = bass_utils.run_bass_kernel_spmd(nc, in_maps,
                                          core_ids=list(range(NCORES)), **kw)
    return res


def kernel(**inputs):
    in_maps, order, chunks = make_in_maps(**inputs)
    res = run(in_maps, chunks)
    ql = np.asarray(inputs["query_lengths"]).astype(np.float32)
    bvp_row = np.asarray(inputs["bvp"], np.float32)
    out = np.zeros((B, DV), np.float32)
    for c in range(NCORES):
        for jj in range(BL):
            e = int(order[8 * jj + c])
            out[e] = res.results[c]["out"][jj] + (ql[e] / LQ) * bvp_row
    return out.astype(np.float32)


# revision 69
# speedup vs baseline: 1.0575x; 1.0575x over previous
"""Trainium2 Bass kernel for MemexQA-FVTA (dense transformer block), v2.

Data-parallel over batch across 8 NeuronCores (8 elems/core, no collectives).

Key differences vs v1 baseline:
- Length-aware tiling: batch elems are sorted by valid token-tile count
  (ceil(text_len/128) + 1 img tile), dealt into 8 per-core slots; each
  slot's tile count is baked into the compiled program (keyed compile
  cache), skipping ~40% of the MACs on average inputs.
- Transposed self-attention softmax: scores computed key-major so the pad
  mask becomes a free per-partition ACT bias and the softmax transposes
  disappear; normalization is deferred (unnormalized album scaled by a
  broadcast 1/rowsum).
- All weights resident in SBUF (no per-elem weight re-DMA); album/keys/
  vals path in bf16 (same PE rate, half the SBUF/DVE traffic).
- LayerNorm rstd via DVE Newton-rsqrt (bitcast seed) - no activation
  table thrash (v1 lost 82us to Ln/Exp table swaps).
- Biases folded: q/k as ACT bias, v as rank-1 PSUM matmuls, bkp as ACT
  bias, bvp applied host-side (exact: sum of FVTA mean weights = ql/LQ).
"""

import sys
import numpy as np

H, B, LT, LI, LQ = 2, 64, 384, 128, 24
D, KD, VD = 768, 384, 384
DV = H * VD          # 768
NCORES = 8
BL = B // NCORES     # 8 batch elements per core
NKC = D // 128       # 6 contraction chunks
MASK = -30000.0
EPS = 1e-5
MAGIC = 0x5F3759DF


def _ensure_path():
    try:
        import concourse  # noqa: F401
    except ImportError:
        sys.path.insert(0, "/opt/trn_rl_repo")


_COMPILED = {}


def build_nc(chunks):
    """Build + compile the per-core Bass program for the given per-slot
    token-tile counts (tuple of 8 ints, each in 2..4). Cached."""
    global _COMPILED
    chunks = tuple(int(c) for c in chunks)
    if chunks in _COMPILED:
        return _COMPILED[chunks]
    _ensure_path()
    from contextlib import ExitStack

    import concourse.bacc as bacc
    import concourse.tile as tile
    import concourse.mybir as mybir

    f32 = mybir.dt.float32
    f32r = mybir.dt.float32r
    bf16 = mybir.dt.bfloat16
    i32 = mybir.dt.int32
    AF = mybir.ActivationFunctionType
    ALU = mybir.AluOpType
    AX = mybir.AxisListType

    TOTC = sum(chunks)
    OFF = [0]
    for c in chunks:
        OFF.append(OFF[-1] + c)

    nc = bacc.Bacc("TRN2", target_bir_lowering=False, debug=False,
                   num_devices=NCORES)

    def din(name, shape, dt=None):
        return nc.declare_dram_parameter(name, list(shape), dt or f32r, False).ap()

    text_d = din("text", [BL, LT, D])
    images_d = din("images", [BL, LI, D])
    query_d = din("query", [BL, LQ, D])
    # packed proj weights: [H, 128, NKC*384]; [h, p, kc*384+m] = W[h, kc*128+p, m]
    w_d = {n: din(n, [H, 128, NKC * 384])
           for n in ["wsq", "wiq", "wsk", "wik", "wsv", "wiv"]}
    wkp_d = din("wkp", [128, NKC * DV], bf16)   # [p, dkc*768+vd] = Wkp[vd,dkc*128+p]
    wvp_d = din("wvp", [128, 12 * 384], bf16)   # [p,(vdc*2+hf)*384+m]
    bqk_d = din("bqk", [128, 24], f32)          # col=(qk*2+h)*3+mf (+12 img)
    bkp_d = din("bkp_t", [128, 6], bf16)
    bv_si_d = din("bv_si", [1, 4 * VD])         # seg = si*2 + h
    kmT_d = din("kmT", [128, TOTC], f32)        # per-partition key mask cols
    qvalid_d = din("qvalid", [1, BL * LQ], f32)  # [j*LQ+q]: valid/LQ, slot j
    ident_d = din("ident_r", [128, 128])
    ones_d = din("ones_r", [1, 128])
    out_d = nc.declare_dram_parameter("out", [BL, DV], f32, True).ap()

    with tile.TileContext(nc) as tc, ExitStack() as ctx:
        def pool(**kw):
            return ctx.enter_context(tc.tile_pool(**kw))

        cpool = pool(name="const", bufs=1)
        wpool = pool(name="wres", bufs=1)
        lnp = pool(name="ln", bufs=2)
        scrq = pool(name="scrq", bufs=1)
        scr = pool(name="scr", bufs=1)
        stat = pool(name="stat", bufs=3)
        xtp = pool(name="xt", bufs=1)
        qkp = pool(name="qk", bufs=1)
        vp = pool(name="v", bufs=1)
        etp = pool(name="et", bufs=1)
        rbp = pool(name="rb", bufs=2)
        sinvp = pool(name="sinv", bufs=1)
        albp = pool(name="alb", bufs=1)
        valsp = pool(name="vals", bufs=1)
        qkpp = pool(name="qkp24", bufs=1)
        qftp = pool(name="qft", bufs=2)
        smallp = pool(name="sml", bufs=1)
        pmm = pool(name="pmm", bufs=5, space="PSUM")
        ptr = pool(name="ptr", bufs=2, space="PSUM")
        psml = pool(name="psml", bufs=1, space="PSUM")

        def r(ap):
            return ap.bitcast(f32r)

        def p(ap):
            return ap.bitcast(f32)

        # ---- constants ----
        ident = cpool.tile([128, 128], f32r, tag="ident")
        nc.sync.dma_start(ident[:], ident_d[:])
        ones1 = cpool.tile([1, 128], f32r, tag="ones1")
        nc.sync.dma_start(ones1[:], ones_d[:])
        negI = cpool.tile([128, 128], f32, tag="negI")
        nc.gpsimd.memset(negI[:], 0.0)
        nc.gpsimd.affine_select(
            out=negI[:], in_=negI[:], compare_op=ALU.not_equal, fill=MASK,
            base=0, pattern=[[-1, 128]], channel_multiplier=1)
        ones_bf = cpool.tile([128, 1], bf16, tag="onesbf")
        nc.gpsimd.memset(ones_bf[:], 1.0)
        bqk = cpool.tile([128, 24], f32, tag="bqk")
        nc.sync.dma_start(bqk[:], bqk_d[:])
        bkp = cpool.tile([128, 6], bf16, tag="bkp")
        nc.sync.dma_start(bkp[:], bkp_d[:])
        bv_si = cpool.tile([1, 4 * VD], f32r, tag="bvsi")
        nc.sync.dma_start(bv_si[:], bv_si_d[:])
        kmT = cpool.tile([128, TOTC], f32, tag="kmT")
        nc.sync.dma_start(kmT[:], kmT_d[:])
        qvalid = cpool.tile([1, BL * LQ], f32, tag="qvalid")
        nc.sync.dma_start(qvalid[:], qvalid_d[:])

        wt = {}

        # ================= per-elem pipeline =================
        def preamble(j):
            """DMA + LN + transposes for slot j. Returns (qfT, xT)."""
            C = chunks[j]
            N = C * 128
            qtile = scrq.tile([LQ, D], f32r, tag="q")
            nc.sync.dma_start(qtile[:], query_d[j])
            qfT = qftp.tile([128, NKC * LQ], bf16, tag="qfT")
            for kc in range(NKC):
                pt = ptr.tile([128, 128], f32r, tag="tr")
                nc.tensor.transpose(r(pt[:, 0:LQ]),
                                    r(qtile[:, kc * 128:(kc + 1) * 128]),
                                    r(ident[0:LQ, 0:LQ]))
                nc.vector.tensor_copy(qfT[:, kc * LQ:(kc + 1) * LQ],
                                      p(pt[:, 0:LQ]))
            # qk_proj = qf @ Wkp.T [24, 768] and qbias = qf @ bkp [1, 24]
            # (fvta inputs that only need qfT - computed early, off the
            # fvta critical path)
            qkp24 = qkpp.tile([LQ, DV], f32r, tag="qkp24")
            ps_h0 = pmm.tile([128, 512], f32, tag="mm")
            ps_h1 = pmm.tile([128, 512], f32, tag="mm")
            pss = [ps_h0, ps_h1]
            for dkc in range(NKC):
                for hf in range(2):
                    nc.tensor.matmul(
                        pss[hf][0:LQ, 0:384],
                        qfT[:, dkc * LQ:(dkc + 1) * LQ],
                        wkp_s[:, dkc * DV + hf * 384: dkc * DV + hf * 384 + 384],
                        start=(dkc == 0), stop=(dkc == NKC - 1))
            for hf in range(2):
                nc.vector.tensor_copy(qkp24[:, hf * 384:(hf + 1) * 384],
                                      pss[hf][0:LQ, 0:384])
            qkpT = qftp.tile([128, NKC * LQ], bf16, tag="qkpT")
            for vdc in range(NKC):
                pt = ptr.tile([128, 128], f32r, tag="tr")
                nc.tensor.transpose(r(pt[:, 0:LQ]),
                                    r(qkp24[:, vdc * 128:(vdc + 1) * 128]),
                                    r(ident[0:LQ, 0:LQ]))
                nc.vector.tensor_copy(qkpT[:, vdc * LQ:(vdc + 1) * LQ],
                                      p(pt[:, 0:LQ]))
            psQ = psml.tile([128, 512], f32, tag="sml")
            for dkc in range(NKC):
                nc.tensor.matmul(psQ[0:1, 0:LQ], bkp[:, dkc:dkc + 1],
                                 qfT[:, dkc * LQ:(dkc + 1) * LQ],
                                 start=(dkc == 0), stop=(dkc == NKC - 1))
            qbrow = qftp.tile([1, LQ], f32r, tag="qbrow")
            nc.vector.tensor_copy(qbrow[0:1, :], psQ[0:1, 0:LQ])
            xT = xtp.tile([128, NKC * 512], f32r, tag="xT")
            for tt in range(C):
                if tt < C - 1:
                    src = text_d[j, tt * 128:(tt + 1) * 128, :]
                else:
                    src = images_d[j]
                x = lnp.tile([128, D], f32r, tag="x")
                nc.sync.dma_start(x[:], src)
                bnst = stat.tile([128, 12], f32, tag="bnst")
                nc.vector.bn_stats(bnst[:, 0:6], p(x[:, 0:384]))
                nc.vector.bn_stats(bnst[:, 6:12], p(x[:, 384:768]))
                mv = stat.tile([128, 2], f32, tag="mv")
                nc.vector.bn_aggr(mv[:], bnst[:])
                mu = mv[:, 0:1]
                ve = stat.tile([128, 1], f32, tag="ve")
                nc.gpsimd.tensor_scalar_add(ve[:], mv[:, 1:2], EPS)
                # Newton rsqrt: seed from bitcast magic, 2 iterations
                t1 = stat.tile([128, 1], i32, tag="t1")
                nc.vector.tensor_scalar(t1[:], ve[:].bitcast(i32), 1, None,
                                        ALU.logical_shift_right)
                t2 = stat.tile([128, 1], i32, tag="t2")
                nc.vector.tensor_scalar_mul(t2[:], t1[:], -1)
                nc.vector.tensor_scalar_add(t2[:], t2[:], MAGIC)
                y0 = t2[:].bitcast(f32)
                hh = stat.tile([128, 1], f32, tag="hh")
                nc.vector.tensor_scalar_mul(hh[:], ve[:], -0.5)
                z = stat.tile([128, 1], f32, tag="z")
                y1 = stat.tile([128, 1], f32, tag="y1")
                nc.vector.tensor_mul(z[:], y0, y0)
                nc.vector.tensor_mul(z[:], z[:], hh[:])
                nc.vector.tensor_scalar_add(z[:], z[:], 1.5)
                nc.vector.tensor_mul(y1[:], y0, z[:])
                rstd = stat.tile([128, 1], f32, tag="rstd")
                nc.vector.tensor_mul(z[:], y1[:], y1[:])
                nc.vector.tensor_mul(z[:], z[:], hh[:])
                nc.vector.tensor_scalar_add(z[:], z[:], 1.5)
                nc.vector.tensor_mul(rstd[:], y1[:], z[:])
                nma = stat.tile([128, 1], f32, tag="nma")
                nc.gpsimd.tensor_mul(nma[:], mu, rstd[:])
                nmr = stat.tile([128, 1], f32, tag="nmr")
                nc.gpsimd.tensor_scalar_mul(nmr[:], nma[:], -1.0)
                nc.scalar.activation(x[:], p(x[:]), AF.Identity,
                                     bias=nmr[:, 0:1], scale=rstd[:, 0:1])
                for kc in range(NKC):
                    pt = ptr.tile([128, 128], f32r, tag="tr")
                    nc.tensor.transpose(r(pt[:]),
                                        r(x[:, kc * 128:(kc + 1) * 128]),
                                        r(ident[:]))
                    nc.vector.tensor_copy(
                        xT[:, kc * N + tt * 128: kc * N + tt * 128 + 128],
                        p(pt[:]))
            return qkpT, qbrow, xT

        def compute_heads(j, state):
            """q/k/v projections + transposed-softmax attention for slot j.
            Returns bf16 albumT [128, 6*N] (head-concat, 1/s-scaled)."""
            qkpT, qbrow, xT = state
            C = chunks[j]
            N = C * 128
            NTX = (C - 1) * 128
            albumT = albp.tile([128, NKC * 512], bf16, tag="albumT")
            for h in range(H):
                qT = qkp.tile([128, 3 * 512], f32r, tag="qT")
                kT = qkp.tile([128, 3 * 512], f32r, tag="kT")
                for qk, dstT in ((0, qT), (1, kT)):
                    wS = wt[("wsq" if qk == 0 else "wsk", h)]
                    wI = wt[("wiq" if qk == 0 else "wik", h)]
                    if NTX >= 256:
                        # text tokens: feature-major direct (full-rate N)
                        for mf in range(3):
                            ps = pmm.tile([128, 512], f32, tag="mm")
                            for kc in range(NKC):
                                nc.tensor.matmul(
                                    ps[:, 0:NTX],
                                    r(wS[:, kc * 384 + mf * 128:
                                         kc * 384 + mf * 128 + 128]),
                                    r(xT[:, kc * N: kc * N + NTX]),
                                    start=(kc == 0), stop=(kc == NKC - 1))
                            bcol = (qk * 2 + h) * 3 + mf
                            nc.scalar.activation(
                                dstT[:, mf * N: mf * N + NTX],
                                ps[:, 0:NTX], AF.Identity,
                                bias=bqk[:, bcol:bcol + 1])
                        text_tcs = []
                    else:
                        # single text tile: token-major (N=384) + transpose
                        text_tcs = list(range(C - 1))
                    for tc in text_tcs + [C - 1]:
                        istext = tc < C - 1
                        wX = wS if istext else wI
                        ps2 = pmm.tile([128, 512], f32, tag="mm")
                        for kc in range(NKC):
                            nc.tensor.matmul(
                                ps2[:, 0:KD],
                                r(xT[:, kc * N + tc * 128:
                                     kc * N + tc * 128 + 128]),
                                r(wX[:, kc * 384:(kc + 1) * 384]),
                                start=(kc == 0), stop=(kc == NKC - 1))
                        tm = scr.tile([128, KD], f32r, tag="imgtm")
                        nc.vector.tensor_copy(tm[:], ps2[:, 0:KD])
                        for mf in range(3):
                            pt = ptr.tile([128, 128], f32r, tag="tr")
                            nc.tensor.transpose(
                                r(pt[:]), r(tm[:, mf * 128:(mf + 1) * 128]),
                                r(ident[:]))
                            bcol = (0 if istext else 12) + (qk * 2 + h) * 3 + mf
                            nc.scalar.activation(
                                dstT[:, mf * N + tc * 128:
                                     mf * N + tc * 128 + 128], p(pt[:]),
                                AF.Identity, bias=bqk[:, bcol:bcol + 1])
                # v: token-major
                v = vp.tile([128, 4 * VD], bf16, tag="v")
                for tc in range(C):
                    istext = tc < C - 1
                    wV = wt[("wsv" if istext else "wiv", h)]
                    ps = pmm.tile([128, 512], f32, tag="mm")
                    for kc in range(NKC):
                        nc.tensor.matmul(
                            ps[:, 0:VD],
                            r(xT[:, kc * N + tc * 128: kc * N + tc * 128 + 128]),
                            r(wV[:, kc * 384:(kc + 1) * 384]),
                            start=(kc == 0), stop=False)
                    seg = (0 if istext else 2) + h
                    nc.tensor.matmul(ps[:, 0:VD], r(ones1[0:1, 0:128]),
                                     r(bv_si[0:1, seg * VD:(seg + 1) * VD]),
                                     start=False, stop=True)
                    nc.vector.tensor_copy(v[:, tc * VD:(tc + 1) * VD],
                                          ps[:, 0:VD])
                # transposed scores + masked softmax (unnormalized)
                eT = etp.tile([128, 4 * 512], bf16, tag="eT")
                psS = psml.tile([128, 512], f32, tag="sml")
                for kc in range(C):
                    ps = pmm.tile([128, 512], f32, tag="mm")
                    for mf in range(3):
                        nc.tensor.matmul(
                            ps[:, 0:N],
                            r(kT[:, mf * N + kc * 128: mf * N + kc * 128 + 128]),
                            r(qT[:, mf * N:(mf + 1) * N]),
                            start=(mf == 0), stop=(mf == 2))
                    nc.vector.tensor_add(ps[:, kc * 128:(kc + 1) * 128],
                                         ps[:, kc * 128:(kc + 1) * 128],
                                         negI[:])
                    nc.scalar.activation(eT[:, kc * N:(kc + 1) * N],
                                         ps[:, 0:N], AF.Exp,
                                         bias=kmT[:, OFF[j] + kc: OFF[j] + kc + 1])
                    nc.tensor.matmul(psS[0:1, 0:N], ones_bf[:, 0:1],
                                     eT[:, kc * N:(kc + 1) * N],
                                     start=(kc == 0), stop=(kc == C - 1))
                srow = sinvp.tile([1, 512], f32r, tag="srow")
                nc.vector.tensor_copy(srow[0:1, 0:N], psS[0:1, 0:N])
                psR = psml.tile([128, 512], f32, tag="sml")
                nc.tensor.matmul(psR[:, 0:N], r(ones1[0:1, 0:128]),
                                 srow[0:1, 0:N], start=True, stop=True)
                rb = rbp.tile([128, 512], f32, tag="rb")
                nc.vector.reciprocal(rb[:, 0:N], psR[:, 0:N])
                for vdc in range(3):
                    psA = pmm.tile([128, 512], f32, tag="mm")
                    for kc in range(C):
                        nc.tensor.matmul(
                            psA[:, 0:N],
                            v[:, kc * VD + vdc * 128: kc * VD + vdc * 128 + 128],
                            eT[:, kc * N:(kc + 1) * N],
                            start=(kc == 0), stop=(kc == C - 1))
                    nc.vector.tensor_mul(
                        albumT[:, (h * 3 + vdc) * N:(h * 3 + vdc + 1) * N],
                        psA[:, 0:N], rb[:, 0:N])
            return albumT

        def fvta(j, albumT, qkpT, qbrow):
            C = chunks[j]
            N = C * 128
            # w2 key-major: [t, q]; pad mask is a per-partition exp bias
            e2T = smallp.tile([128, 4 * LQ], bf16, tag="e2T")
            psS2 = psml.tile([128, 512], f32, tag="sml")
            for tc in range(C):
                psw = ptr.tile([128, 128], f32r, tag="tr")
                for vdc in range(NKC):
                    nc.tensor.matmul(
                        p(psw[:, 0:LQ]),
                        albumT[:, vdc * N + tc * 128: vdc * N + tc * 128 + 128],
                        qkpT[:, vdc * LQ:(vdc + 1) * LQ],
                        start=(vdc == 0), stop=False)
                nc.tensor.matmul(p(psw[:, 0:LQ]), r(ones1[0:1, 0:128]),
                                 qbrow[0:1, :], start=False, stop=True)
                nc.scalar.activation(e2T[:, tc * LQ:(tc + 1) * LQ],
                                     p(psw[:, 0:LQ]), AF.Exp,
                                     bias=kmT[:, OFF[j] + tc: OFF[j] + tc + 1])
                nc.tensor.matmul(psS2[0:1, 0:LQ], ones_bf[:, 0:1],
                                 e2T[:, tc * LQ:(tc + 1) * LQ],
                                 start=(tc == 0), stop=(tc == C - 1))
            r2 = smallp.tile([1, LQ], f32, tag="r2")
            nc.vector.reciprocal(r2[0:1, :], psS2[0:1, 0:LQ])
            wp_row = smallp.tile([1, LQ], f32r, tag="wprow")
            nc.vector.tensor_mul(wp_row[0:1, :],
                                 qvalid[0:1, j * LQ:(j + 1) * LQ],
                                 r2[0:1, :])
            pswb = psml.tile([128, 512], f32, tag="sml")
            nc.tensor.matmul(pswb[:, 0:LQ], r(ones1[0:1, 0:128]),
                             wp_row[0:1, :], start=True, stop=True)
            wpB = smallp.tile([128, LQ], f32, tag="wpB")
            nc.vector.tensor_copy(wpB[:], pswb[:, 0:LQ])
            macc = smallp.tile([128, 4], f32, tag="macc")
            mtmp = smallp.tile([128, LQ], f32, tag="mtmp")
            for tc in range(C):
                nc.vector.tensor_mul(mtmp[:], e2T[:, tc * LQ:(tc + 1) * LQ],
                                     wpB[:])
                nc.vector.reduce_sum(macc[:, tc:tc + 1], mtmp[:], axis=AX.X)
            mbarT = smallp.tile([128, 4], bf16, tag="mbarT")
            nc.vector.tensor_copy(mbarT[:, 0:C], macc[:, 0:C])
            vals = valsp.tile([128, 4 * DV], bf16, tag="vals")
            for tc in range(C):
                ps_f0 = pmm.tile([128, 512], f32, tag="mm")
                ps_f1 = pmm.tile([128, 512], f32, tag="mm")
                psf = [ps_f0, ps_f1]
                for vdc in range(NKC):
                    for hf in range(2):
                        nc.tensor.matmul(
                            psf[hf][:, 0:384],
                            albumT[:, vdc * N + tc * 128:
                                   vdc * N + tc * 128 + 128],
                            wvp_s[:, (vdc * 2 + hf) * 384:
                                  (vdc * 2 + hf + 1) * 384],
                            start=(vdc == 0), stop=(vdc == NKC - 1))
                for hf in range(2):
                    nc.vector.tensor_copy(
                        vals[:, tc * DV + hf * 384: tc * DV + hf * 384 + 384],
                        psf[hf][:, 0:384])
            for hf in range(2):
                pso = psml.tile([128, 512], f32, tag="sml")
                for tc in range(C):
                    nc.tensor.matmul(
                        pso[0:1, 0:384], mbarT[:, tc:tc + 1],
                        vals[:, tc * DV + hf * 384: tc * DV + hf * 384 + 384],
                        start=(tc == 0), stop=(tc == C - 1))
                outrow = smallp.tile([1, 384], f32, tag="outrow")
                nc.vector.tensor_copy(outrow[0:1, :], pso[0:1, 0:384])
                nc.sync.dma_start(out_d[j:j + 1, hf * 384:(hf + 1) * 384],
                                  outrow[0:1, :])

        # PE warmup: keep the HAM activity window busy while input/weight
        # DMAs land, so the first real matmuls run at 2.4 GHz
        for _ in range(24):
            ptw = ptr.tile([128, 128], f32r, tag="tr")
            nc.tensor.transpose(r(ptw[:]), r(ident[:]), r(ident[:]))

        wkp_s = wpool.tile([128, NKC * DV], bf16, tag="wkp")
        nc.sync.dma_start(wkp_s[:], wkp_d[:])

        state = preamble(0)

        # resident weights emitted after slot-0 preamble so its input DMAs
        # aren't queued behind ~17MB of weights; h-major, q/k first
        for h in range(H):
            for name in ["wsq", "wsk", "wiq", "wik", "wsv", "wiv"]:
                t = wpool.tile([128, NKC * 384], f32r, tag=f"{name}{h}")
                nc.sync.dma_start(t[:], w_d[name][h])
                wt[(name, h)] = t
        wvp_s = wpool.tile([128, 12 * 384], bf16, tag="wvp")
        nc.sync.dma_start(wvp_s[:], wvp_d[:])

        for j in range(BL):
            qkpT, qbrow, xT = state
            albumT = compute_heads(j, state)
            if j + 1 < BL:
                state = preamble(j + 1)
            fvta(j, albumT, qkpT, qbrow)

    nc.compile()
    _COMPILED[chunks] = nc
    return nc


def plan_slots(text_lengths):
    """Sort elems by token-tile count desc, deal into 8 slots x 8 cores.
    Returns (order[64], chunks[8]): core c slot j processes elem
    order[8*j + c]; chunks[j] = baked tile count for slot j."""
    tl = np.asarray(text_lengths)
    ntt = np.clip(np.ceil(tl / 128).astype(np.int64), 1, 3)
    order = np.argsort(-ntt, kind="stable")
    chunks = [int(ntt[order[8 * jj]]) + 1 for jj in range(BL)]
    return order, chunks


def make_in_maps(text, images, query, ln_gamma, ln_beta,
                 Wsq, bsq, Wiq, biq, Wsk, bsk, Wik, bik, Wsv, bsv, Wiv, biv,
                 Wkp, bkp, Wvp, bvp,
                 text_lengths, image_lengths, query_lengths):
    """Host-side preprocessing + slot-sorted batch sharding."""
    _ensure_path()
    import ml_dtypes
    f = np.float32
    g = np.asarray(ln_gamma, f)
    beta = np.asarray(ln_beta, f)

    order, chunks = plan_slots(text_lengths)
    TOTC = sum(chunks)
    OFF = [0]
    for c in chunks:
        OFF.append(OFF[-1] + c)

    def fold_w(W):
        return np.asarray(W, f) * g[None, :, None]

    def pack_w(W):
        M = W.shape[2]
        return np.ascontiguousarray(
            W.reshape(H, NKC, 128, M).transpose(0, 2, 1, 3).reshape(H, 128, NKC * M))

    def beta_bias(W, bias):
        Wf = fold_w(W)
        return (np.einsum("d,hdm->hm", beta, Wf) + np.asarray(bias, f)).astype(f)

    ws = {}
    for name, W in [("wsq", Wsq), ("wiq", Wiq), ("wsk", Wsk), ("wik", Wik),
                    ("wsv", Wsv), ("wiv", Wiv)]:
        ws[name] = pack_w(fold_w(W))
    bq_s = beta_bias(Wsq, bsq)
    bk_s = beta_bias(Wsk, bsk)
    bv_s = beta_bias(Wsv, bsv)
    bq_i = beta_bias(Wiq, biq)
    bk_i = beta_bias(Wik, bik)
    bv_i = beta_bias(Wiv, biv)

    # bqk [128, 24]: col=(qk*2+h)*3+mf text, +12 img
    bqk = np.zeros((128, 24), f)
    for qk, (bt, bi) in enumerate([(bq_s, bq_i), (bk_s, bk_i)]):
        for h in range(H):
            for mf in range(3):
                col = (qk * 2 + h) * 3 + mf
                bqk[:, col] = bt[h, mf * 128:(mf + 1) * 128]
                bqk[:, 12 + col] = bi[h, mf * 128:(mf + 1) * 128]
    bv_si = np.concatenate([bv_s[0], bv_s[1], bv_i[0], bv_i[1]]).astype(f)
    bv_si = bv_si.reshape(1, 4 * VD)

    Wkp_ = np.asarray(Wkp, f)
    # [p, dkc*768 + vd] = Wkp[vd, dkc*128+p]
    wkp_p = np.ascontiguousarray(
        Wkp_.reshape(DV, NKC, 128).transpose(2, 1, 0).reshape(128, NKC * DV))
    Wvp_ = np.asarray(Wvp, f)
    wvp_p = np.ascontiguousarray(
        Wvp_.reshape(NKC, 128, 2, 384).transpose(1, 0, 2, 3).reshape(128, 12 * 384))
    bkp_t = np.ascontiguousarray(np.asarray(bkp, f).reshape(6, 128).T)

    tl = np.asarray(text_lengths)
    il = np.asarray(image_lengths)
    ql = np.asarray(query_lengths)

    def rnd(a):
        a = np.ascontiguousarray(np.asarray(a, f))
        return (a.view(np.uint32) & np.uint32(0xFFFFF000)).view(np.float32)

    ident_r = rnd(np.eye(128, dtype=f))
    ones_r = rnd(np.ones((1, 128), f))
    text = rnd(np.asarray(text, f))
    images = rnd(np.asarray(images, f))
    query = rnd(np.asarray(query, f))
    for n in list(ws):
        ws[n] = rnd(ws[n])
    bv_si = rnd(bv_si)
    wkp_b = wkp_p.astype(ml_dtypes.bfloat16)
    wvp_b = wvp_p.astype(ml_dtypes.bfloat16)
    bkp_b = bkp_t.astype(ml_dtypes.bfloat16)

    in_maps = []
    for c in range(NCORES):
        el = [int(order[8 * jj + c]) for jj in range(BL)]
        kmT = np.zeros((128, TOTC), f)
        qvalid_r = np.zeros((1, BL * LQ), f)
        for jj in range(BL):
            e = el[jj]
            C = chunks[jj]
            km = np.zeros(C * 128, f)
            ntx = (C - 1) * 128
            km[:ntx][np.arange(ntx) >= tl[e]] = MASK
            km[ntx:][np.arange(128) >= il[e]] = MASK
            for cc in range(C):
                kmT[:, OFF[jj] + cc] = km[cc * 128:(cc + 1) * 128]
            qvalid_r[0, jj * LQ:(jj + 1) * LQ] = \
                (np.arange(LQ) < ql[e]).astype(f) / LQ
        in_maps.append({
            "text": np.ascontiguousarray(text[el]),
            "images": np.ascontiguousarray(images[el]),
            "query": np.ascontiguousarray(query[el]),
            **{n: ws[n] for n in ws},
            "wkp": wkp_b, "wvp": wvp_b,
            "bqk": bqk, "bkp_t": bkp_b, "bv_si": bv_si,
            "kmT": kmT, "qvalid": qvalid_r,
            "ident_r": ident_r, "ones_r": ones_r,
        })
    return in_maps, order, chunks


def run(in_maps, chunks, trace=False, tmpdir=None):
    _ensure_path()
    from concourse import bass_utils
    nc = build_nc(chunks)
    kw = {}
    if trace:
        kw = dict(trace=True, tmpdir=tmpdir)
    res = bass_utils.run_bass_kernel_spmd(nc, in_maps,
                                          core_ids=list(range(NCORES)), **kw)
    return res


def kernel(**inputs):
    in_maps, order, chunks = make_in_maps(**inputs)
    res = run(in_maps, chunks)
    ql = np.asarray(inputs["query_lengths"]).astype(np.float32)
    bvp_row = np.asarray(inputs["bvp"], np.float32)
    out = np.zeros((B, DV), np.float32)
    for c in range(NCORES):
        for jj in range(BL):
            e = int(order[8 * jj + c])
            out[e] = res.results[c]["out"][jj] + (ql[e] / LQ) * bvp_row
    return out.astype(np.float32)
